# revision 14
# baseline (speedup 1.0000x reference)
"""Trainium2 Bass kernel for LoRALinear: out = x @ W.T + b + scale*(x @ A.T) @ B.T.

Strategy
--------
* 8-way data-parallel over the flattened (batch*seq) rows: 16384 rows -> 2048
  rows per NeuronCore.  W / lora weights are replicated; no collectives.
* On-chip each core computes the transposed output block
      outT = W @ x_shard.T  (+ tail)            [out_f, rows]
  so the stationary matmul operand is a 128x128 W-block and the moving
  operand is a [128, 512] x.T tile (x.T is fully SBUF-resident in bf16).
* The LoRA low-rank path and the bias are folded into the same PSUM
  accumulation as one extra "tail" matmul per output tile:
      rows 0..7  of tail lhsT = scale * B.T     (contracted with xaT)
      row  8     of tail lhsT = b               (contracted with a ones row)
      rows 9..127 zero
  where xaT = A @ x.T is computed on-device first (tiny matmul).
* All matmuls are bf16 inputs / fp32 PSUM accumulation.
* Host side: shard + pre-layout (transpose/cast) inputs, transpose outputs
  back.  Only the NEFF execution happens on device.
"""

import numpy as np
import ml_dtypes

import concourse.bass as bass
import concourse.bacc as bacc_mod
import concourse.mybir as mybir
import concourse.tile as tile
from concourse.bass_utils import run_bass_kernel_spmd

N_CORES = 8
P = 128
RF = 512  # moving free dim per matmul

IN_F = 4096
OUT_F = 4096
RANK = 8
BIAS_ROW = 32  # partition carrying the all-ones bias row in xa_sb
SCALE = 8.0 / 8.0  # alpha / rank
B_DIM = 4
S_DIM = 4096
ROWS_TOTAL = B_DIM * S_DIM
ROWS = ROWS_TOTAL // N_CORES

BF16 = mybir.dt.bfloat16
F32 = mybir.dt.float32
NP_BF16 = ml_dtypes.bfloat16


def _build(rows, in_f, out_f):
    """Build the per-core Bass program (same program for all cores)."""
    ko = in_f // P   # contraction subtiles
    nb = out_f // P  # output-feature blocks (psum partition dim)
    rb = rows // RF  # row chunks (moving free dim)

    nc = bacc_mod.Bacc()
    xprep = nc.declare_dram_parameter("xprep", [P, ko, rows], BF16, isOutput=False)
    wprep = nc.declare_dram_parameter("wprep", [nb, P, ko, P], BF16, isOutput=False)
    aprep = nc.declare_dram_parameter("aprep", [P, ko, RANK], BF16, isOutput=False)
    tailprep = nc.declare_dram_parameter("tailprep", [P, out_f], BF16, isOutput=False)
    outT = nc.declare_dram_parameter("outT", [out_f, rows], F32, isOutput=True)

    with tile.TileContext(nc) as tc:
        with (
            tc.tile_pool(name="const", bufs=1) as const,
            tc.tile_pool(name="xpool", bufs=1) as xpool,
            tc.tile_pool(name="wpool", bufs=3) as wpool,
            tc.tile_pool(name="opool", bufs=4) as opool,
            tc.tile_pool(name="mpsum", bufs=6, space="PSUM") as mpsum,
            tc.tile_pool(name="xapsum", bufs=2, space="PSUM") as xapsum,
        ):
            a_sb = const.tile([P, ko, RANK], BF16)
            nc.sync.dma_start(a_sb, aprep[:])
            tail_sb = const.tile([P, out_f], BF16)
            nc.sync.dma_start(tail_sb, tailprep[:])

            # xa_sb rows 0..7 = xaT (filled below), row BIAS_ROW = ones (bias
            # row, at partition 32 because compute-engine writes must start at
            # a 32-aligned partition), all other rows zero so the 128-deep
            # tail matmul adds nothing.
            xa_sb = const.tile([P, rows], BF16)
            nc.vector.memset(xa_sb, 0.0)
            nc.vector.memset(xa_sb[BIAS_ROW : BIAS_ROW + 1, :], 1.0)

            # x.T resident in SBUF, loaded per contraction subtile so compute
            # can start before the whole tensor has arrived.
            x_sb = xpool.tile([P, ko, rows], BF16)
            for k in range(ko):
                nc.sync.dma_start(x_sb[:, k], xprep[:, k])

            # Stage A: xaT = A @ x.T  -> [RANK, rows]
            for r in range(rb):
                pxa = xapsum.tile([RANK, RF], F32, name="pxa", tag="pxa")
                for k in range(ko):
                    nc.tensor.matmul(
                        pxa,
                        lhsT=a_sb[:, k],
                        rhs=x_sb[:, k, r * RF : (r + 1) * RF],
                        start=(k == 0),
                        stop=(k == ko - 1),
                    )
                nc.vector.tensor_copy(
                    out=xa_sb[:RANK, r * RF : (r + 1) * RF], in_=pxa
                )

            # Main: outT[n] = W_n @ x.T (+ tail), accumulated over ko k-tiles.
            for n in range(nb):
                w_sb = wpool.tile([P, ko, P], BF16, name="w_sb", tag="w_sb")
                nc.sync.dma_start(w_sb, wprep[n])
                psums = [
                    mpsum.tile([P, RF], F32, name="ps", tag="ps") for _ in range(rb)
                ]
                for k in range(ko):
                    for r in range(rb):
                        nc.tensor.matmul(
                            psums[r],
                            lhsT=w_sb[:, k],
                            rhs=x_sb[:, k, r * RF : (r + 1) * RF],
                            start=(k == 0),
                            stop=False,
                        )
                for r in range(rb):
                    nc.tensor.matmul(
                        psums[r],
                        lhsT=tail_sb[:, n * P : (n + 1) * P],
                        rhs=xa_sb[:, r * RF : (r + 1) * RF],
                        start=False,
                        stop=True,
                    )
                    o_sb = opool.tile([P, RF], F32, name="o_sb", tag="o_sb")
                    nc.vector.tensor_copy(out=o_sb, in_=psums[r])
                    nc.sync.dma_start(
                        outT[n * P : (n + 1) * P, r * RF : (r + 1) * RF], o_sb
                    )
    nc.finalize()
    return nc


def _prep_shared(W, b, lora_A, lora_B, in_f, out_f):
    ko = in_f // P
    nb = out_f // P
    # wprep[n, ki, ko_, o] = W[n*128+o, ko_*128+ki]
    wprep = W.T.reshape(ko, P, nb, P).transpose(2, 1, 0, 3).astype(NP_BF16)
    aprep = lora_A.T.reshape(ko, P, RANK).transpose(1, 0, 2).astype(NP_BF16)
    tail = np.zeros((P, out_f), np.float32)
    tail[:RANK] = SCALE * lora_B.T
    tail[BIAS_ROW] = b
    tailprep = tail.astype(NP_BF16)
    return wprep, aprep, tailprep


def _prep_x_shard(x2d, core, rows, in_f):
    ko = in_f // P
    xs = x2d[core * rows : (core + 1) * rows]
    # xprep[ki, ko_, r] = xs[r, ko_*128+ki]
    return xs.T.reshape(ko, P, rows).transpose(1, 0, 2).astype(NP_BF16)


def _prepare(x, W, b, lora_A, lora_B):
    """Build the Bass module and per-core input maps for these inputs."""
    x = np.asarray(x, np.float32)
    W = np.asarray(W, np.float32)
    b = np.asarray(b, np.float32)
    lora_A = np.asarray(lora_A, np.float32)
    lora_B = np.asarray(lora_B, np.float32)

    rows_total = x.shape[0] * x.shape[1] if x.ndim == 3 else x.shape[0]
    in_f = x.shape[-1]
    out_f = W.shape[0]
    rows = rows_total // N_CORES
    x2d = np.ascontiguousarray(x.reshape(rows_total, in_f))

    nc = _build(rows, in_f, out_f)
    wprep, aprep, tailprep = _prep_shared(W, b, lora_A, lora_B, in_f, out_f)
    in_maps = []
    for c in range(N_CORES):
        in_maps.append(
            {
                "xprep": _prep_x_shard(x2d, c, rows, in_f),
                "wprep": wprep,
                "aprep": aprep,
                "tailprep": tailprep,
            }
        )
    return nc, in_maps, (rows_total, rows, out_f, x.shape)


def _run(x, W, b, lora_A, lora_B, trace=False, trace_kwargs=None):
    nc, in_maps, (rows_total, rows, out_f, xshape) = _prepare(
        x, W, b, lora_A, lora_B
    )

    kwargs = {}
    if trace:
        kwargs["trace"] = True
        if trace_kwargs:
            kwargs["trace_kwargs"] = trace_kwargs
    res = run_bass_kernel_spmd(nc, in_maps, list(range(N_CORES)), **kwargs)

    out = np.empty((rows_total, out_f), np.float32)
    for c in range(N_CORES):
        out[c * rows : (c + 1) * rows] = res.results[c]["outT"].T
    if len(xshape) == 3:
        out = out.reshape(xshape[0], xshape[1], out_f)
    return out, res


def kernel(x, W, b, lora_A, lora_B):
    out, _ = _run(x, W, b, lora_A, lora_B, trace=False)
    return out
